# revision 5
# baseline (speedup 1.0000x reference)
"""BinaryMLP (dense_mlp) Trainium2 kernel — 8-core data-parallel sync-BN.

Strategy:
  - Shard batch (4096) across 8 NeuronCores (512 rows each); replicate weights.
  - Activations live in SBUF transposed: [features -> partitions, batch -> free].
    BatchNorm batch stats are then free-axis reductions (VectorE / ACT accum).
  - Matmuls: lhsT = W.T tile (stationary), rhs = xT tile (moving), bf16 in,
    fp32 PSUM accumulation. Weight sign() / transposes / casts done host-side.
  - Sync-BN: per-feature sum / sumsq reduced locally, then one 32KB fp32
    AllReduce per BN layer across the 8 cores.
  - Final Linear flips layout back to [batch -> partitions, classes -> free] by
    using the h3 activation tiles as the stationary operand; log_softmax is a
    free-axis max/exp-accum/ln chain; b3 is folded in via an extra
    ones-row x b3-row contraction tile.
"""

import os
import sys

for _p in ("/opt/trn_rl_repo",):
    if _p not in sys.path and os.path.isdir(_p):
        sys.path.insert(0, _p)

import numpy as np
import ml_dtypes

import concourse.bass as bass
import concourse.mybir as mybir
import concourse.tile as tile
from concourse import bacc
from concourse.bass_utils import run_bass_kernel_spmd

AF = mybir.ActivationFunctionType
ALU = mybir.AluOpType
F32 = mybir.dt.float32
BF16 = mybir.dt.bfloat16
AX = mybir.AxisListType

NP_BF16 = ml_dtypes.bfloat16

P = 128
N_CORES = 8
B_TOTAL = 4096
D_IN = 4096
H1, H2, H3 = 4096, 4096, 2048
C = 1000
BN_EPS = 1e-5

KT0, MT0 = D_IN // P, H1 // P  # 32, 32
KT1, MT1 = H1 // P, H2 // P  # 32, 32
KT2, MT2 = H2 // P, H3 // P  # 32, 16
KT3 = H3 // P  # 16 (+1 aug tile for the bias)
GROUP = 8  # out-feature tiles per PSUM group


def build(b_shard: int, n_cores: int):
    """Build + compile the SPMD program for a per-core batch shard of b_shard."""
    assert b_shard % P == 0
    nb = b_shard // P  # batch tiles for the final layer
    n_batch_global = b_shard * n_cores
    rg = [list(range(n_cores))]

    nc = bacc.Bacc(
        "TRN2", target_bir_lowering=False, debug=False, num_devices=n_cores
    )

    xT = nc.dram_tensor("xT", [D_IN, b_shard], BF16, kind="ExternalInput").ap()
    w0t = nc.dram_tensor("w0t", [D_IN, H1], BF16, kind="ExternalInput").ap()
    w1t = nc.dram_tensor("w1t", [H1, H2], BF16, kind="ExternalInput").ap()
    w2t = nc.dram_tensor("w2t", [H2, H3], BF16, kind="ExternalInput").ap()
    # W3.T augmented with a b3 row (row H3) + zero padding to a full k-tile.
    w3t = nc.dram_tensor("w3t", [(KT3 + 1) * P, C], BF16, kind="ExternalInput").ap()
    g0p = nc.dram_tensor("g0p", [P, MT0], F32, kind="ExternalInput").ap()
    b0p = nc.dram_tensor("b0p", [P, MT0], F32, kind="ExternalInput").ap()
    g1p = nc.dram_tensor("g1p", [P, MT1], F32, kind="ExternalInput").ap()
    b1p = nc.dram_tensor("b1p", [P, MT1], F32, kind="ExternalInput").ap()
    g2p = nc.dram_tensor("g2p", [P, MT2], F32, kind="ExternalInput").ap()
    b2p = nc.dram_tensor("b2p", [P, MT2], F32, kind="ExternalInput").ap()
    out = nc.dram_tensor("out", [b_shard, C], F32, kind="ExternalOutput").ap()

    with tile.TileContext(nc) as tc:
        with (
            tc.tile_pool(name="big", bufs=1) as big,
            tc.tile_pool(name="wpool", bufs=4) as wpool,
            tc.tile_pool(name="psum", bufs=8, space="PSUM") as psum,
            tc.tile_pool(name="scratch", bufs=4) as scratch,
            tc.tile_pool(name="bn", bufs=8) as bnp,
            tc.tile_pool(name="small", bufs=24) as small,
            tc.tile_pool(name="dram", bufs=1, space="DRAM") as dram,
        ):
            # ---- persistent activation buffers -------------------------------
            xT_sb = big.tile([P, KT0, b_shard], BF16, name="xT_sb", tag="xT_sb")
            h1_sb = big.tile([P, MT0, b_shard], BF16, name="h1_sb", tag="h1_sb")
            h2_sb = big.tile([P, MT1, b_shard], BF16, name="h2_sb", tag="h2_sb")
            h3_sb = big.tile([P, MT2, b_shard], BF16, name="h3_sb", tag="h3_sb")
            ones_t = big.tile([P, b_shard], BF16, name="ones_t", tag="ones_t")

            nc.sync.dma_start(
                xT_sb[:], xT.rearrange("(ko p) b -> p ko b", p=P)
            )
            nc.gpsimd.memset(ones_t[:], 0.0)
            nc.gpsimd.memset(ones_t[:1, :], 1.0)

            # BN gamma/beta (host packed to [P, MT])
            gb = {}
            for nm, ap_, mt in (
                ("g0", g0p, MT0),
                ("b0", b0p, MT0),
                ("g1", g1p, MT1),
                ("b1", b1p, MT1),
                ("g2", g2p, MT2),
                ("b2", b2p, MT2),
            ):
                t = big.tile([P, mt], F32, name=f"{nm}_sb", tag=f"{nm}_sb")
                nc.sync.dma_start(t[:], ap_)
                gb[nm] = t

            def mlp_layer(lidx, in_sb, kt, mt, w_dram, g_sb, b_sb, out_sb):
                """out_sb <- relu(bn(in_sb.T @ W.T)) in transposed layout."""
                stats = big.tile(
                    [P, 2 * mt], F32, name=f"stats{lidx}", tag=f"stats{lidx}"
                )
                gstats = big.tile(
                    [P, 2 * mt], F32, name=f"gstats{lidx}", tag=f"gstats{lidx}"
                )
                arin = dram.tile([P, 2 * mt], F32, name=f"arin{lidx}", tag=f"arin{lidx}")
                arout = dram.tile(
                    [P, 2 * mt], F32, name=f"arout{lidx}", tag=f"arout{lidx}"
                )

                for g in range(mt // GROUP):
                    ps = [
                        psum.tile([P, b_shard], F32, name=f"ps{lidx}_{g}_{j}", tag="ps")
                        for j in range(GROUP)
                    ]
                    for k in range(kt):
                        slab = wpool.tile(
                            [P, GROUP * P], BF16, name=f"w{lidx}_{g}_{k}", tag="wslab"
                        )
                        nc.sync.dma_start(
                            slab[:],
                            w_dram[
                                k * P : (k + 1) * P,
                                g * GROUP * P : (g + 1) * GROUP * P,
                            ],
                        )
                        for j in range(GROUP):
                            nc.tensor.matmul(
                                ps[j][:],
                                slab[:, j * P : (j + 1) * P],
                                in_sb[:, k, :],
                                start=(k == 0),
                                stop=(k == kt - 1),
                            )
                    for j in range(GROUP):
                        m = g * GROUP + j
                        # h_pre -> SBUF bf16 (kept for the post-AR BN apply)
                        nc.scalar.activation(out_sb[:, m, :], ps[j][:], AF.Copy)
                        # local batch stats (fp32, straight from PSUM)
                        nc.vector.tensor_reduce(
                            stats[:, m : m + 1], ps[j][:], axis=AX.X, op=ALU.add
                        )
                        sq = scratch.tile(
                            [P, b_shard], F32, name=f"sq{lidx}_{m}", tag="sq"
                        )
                        # HW: only one PSUM read per DVE inst, so square on ACT
                        nc.scalar.activation(
                            sq[:],
                            ps[j][:],
                            AF.Square,
                            accum_out=stats[:, mt + m : mt + m + 1],
                        )

                # ---- sync-BN all-reduce (2*mt fp32 per feature row) ----------
                nc.gpsimd.dma_start(arin[:], stats[:])
                nc.gpsimd.collective_compute(
                    "AllReduce",
                    ALU.add,
                    replica_groups=rg,
                    ins=[arin.opt()],
                    outs=[arout.opt()],
                )
                nc.gpsimd.dma_start(gstats[:], arout[:])

                # ---- scale/shift: s = g * rsqrt(var+eps); t = beta - mean*s --
                inv_n = 1.0 / float(n_batch_global)
                mean = bnp.tile([P, mt], F32, name=f"mean{lidx}", tag="bn")
                ex2 = bnp.tile([P, mt], F32, name=f"ex2{lidx}", tag="bn")
                m2 = bnp.tile([P, mt], F32, name=f"m2{lidx}", tag="bn")
                var = bnp.tile([P, mt], F32, name=f"var{lidx}", tag="bn")
                inv = bnp.tile([P, mt], F32, name=f"inv{lidx}", tag="bn")
                rstd = bnp.tile([P, mt], F32, name=f"rstd{lidx}", tag="bn")
                tmp = bnp.tile([P, mt], F32, name=f"tmp{lidx}", tag="bn")
                s_sb = big.tile([P, mt], F32, name=f"s{lidx}", tag=f"s{lidx}")
                t_sb = big.tile([P, mt], F32, name=f"t{lidx}", tag=f"t{lidx}")

                nc.scalar.activation(mean[:], gstats[:, :mt], AF.Copy, scale=inv_n)
                nc.scalar.activation(ex2[:], gstats[:, mt:], AF.Copy, scale=inv_n)
                nc.vector.tensor_mul(m2[:], mean[:], mean[:])
                nc.vector.tensor_sub(var[:], ex2[:], m2[:])
                nc.vector.tensor_scalar_add(var[:], var[:], BN_EPS)
                nc.vector.reciprocal(inv[:], var[:])
                nc.scalar.activation(rstd[:], inv[:], AF.Sqrt)
                nc.vector.tensor_mul(s_sb[:], rstd[:], g_sb[:])
                nc.vector.tensor_mul(tmp[:], mean[:], s_sb[:])
                nc.vector.tensor_sub(t_sb[:], b_sb[:], tmp[:])

                # ---- apply BN + ReLU in place --------------------------------
                for m in range(mt):
                    nc.scalar.activation(
                        out_sb[:, m, :],
                        out_sb[:, m, :],
                        AF.Relu,
                        bias=t_sb[:, m : m + 1],
                        scale=s_sb[:, m : m + 1],
                    )

            mlp_layer(0, xT_sb, KT0, MT0, w0t, gb["g0"], gb["b0"], h1_sb)
            mlp_layer(1, h1_sb, KT1, MT1, w1t, gb["g1"], gb["b1"], h2_sb)
            mlp_layer(2, h2_sb, KT2, MT2, w2t, gb["g2"], gb["b2"], h3_sb)

            # ---- final Linear + log_softmax ---------------------------------
            # lhsT = h3 tile slice (stationary), rhs = W3.T slab (moving).
            # Output layout flips to [batch -> partitions, classes -> free].
            half = (C + 1) // 2  # 500
            ps3 = [
                [
                    psum.tile([P, 512], F32, name=f"ps3_{b}_{h}", tag="ps")
                    for h in range(2)
                ]
                for b in range(nb)
            ]
            for k in range(KT3 + 1):
                slab = wpool.tile([P, GROUP * P], BF16, name=f"w3_{k}", tag="wslab")
                nc.sync.dma_start(slab[:, :C], w3t[k * P : (k + 1) * P, :])
                for b in range(nb):
                    lhsT = (
                        h3_sb[:, k, b * P : (b + 1) * P]
                        if k < KT3
                        else ones_t[:, b * P : (b + 1) * P]
                    )
                    for h in range(2):
                        nc.tensor.matmul(
                            ps3[b][h][:, : half],
                            lhsT,
                            slab[:, h * half : (h + 1) * half],
                            start=(k == 0),
                            stop=(k == KT3),
                        )

            for b in range(nb):
                p0 = ps3[b][0][:, :half]
                p1 = ps3[b][1][:, :half]
                m0 = small.tile([P, 1], F32, name=f"m0_{b}", tag="sm")
                m1 = small.tile([P, 1], F32, name=f"m1_{b}", tag="sm")
                nmax = small.tile([P, 1], F32, name=f"nmax_{b}", tag="sm")
                s0 = small.tile([P, 1], F32, name=f"s0_{b}", tag="sm")
                s1 = small.tile([P, 1], F32, name=f"s1_{b}", tag="sm")
                ssum = small.tile([P, 1], F32, name=f"ssum_{b}", tag="sm")
                lse = small.tile([P, 1], F32, name=f"lse_{b}", tag="sm")
                shift = small.tile([P, 1], F32, name=f"shift_{b}", tag="sm")

                nc.vector.tensor_reduce(m0[:], p0, axis=AX.X, op=ALU.max)
                nc.vector.tensor_reduce(m1[:], p1, axis=AX.X, op=ALU.max)
                nc.vector.tensor_max(m0[:], m0[:], m1[:])
                nc.vector.tensor_scalar_mul(nmax[:], m0[:], -1.0)
                e0 = scratch.tile([P, 512], F32, name=f"e0_{b}", tag="sq")
                e1 = scratch.tile([P, 512], F32, name=f"e1_{b}", tag="sq")
                nc.scalar.activation(
                    e0[:, :half], p0, AF.Exp, bias=nmax[:], scale=1.0, accum_out=s0[:]
                )
                nc.scalar.activation(
                    e1[:, :half], p1, AF.Exp, bias=nmax[:], scale=1.0, accum_out=s1[:]
                )
                nc.vector.tensor_add(ssum[:], s0[:], s1[:])
                nc.scalar.activation(lse[:], ssum[:], AF.Ln)
                nc.vector.tensor_sub(shift[:], nmax[:], lse[:])
                o0 = scratch.tile([P, 512], F32, name=f"o0_{b}", tag="sq")
                o1 = scratch.tile([P, 512], F32, name=f"o1_{b}", tag="sq")
                nc.scalar.activation(
                    o0[:, :half], p0, AF.Identity, bias=shift[:], scale=1.0
                )
                nc.scalar.activation(
                    o1[:, :half], p1, AF.Identity, bias=shift[:], scale=1.0
                )
                nc.sync.dma_start(out[b * P : (b + 1) * P, :half], o0[:, :half])
                nc.sync.dma_start(out[b * P : (b + 1) * P, half:C], o1[:, :half])

    nc.compile()
    return nc


def prep_inputs(inputs, b_shard: int, n_cores: int):
    """Host-side prep: shard x, transpose/cast weights, pack BN params."""
    x = np.ascontiguousarray(inputs["x"], dtype=np.float32)

    def bf(a):
        return np.ascontiguousarray(a).astype(NP_BF16)

    def sign_f32(w):
        return np.where(w >= 0, np.float32(1.0), np.float32(-1.0))

    w0t = bf(inputs["W0"].astype(np.float32).T)
    w1t = bf(sign_f32(np.asarray(inputs["Wb1"], dtype=np.float32)).T)
    w2t = bf(sign_f32(np.asarray(inputs["Wb2"], dtype=np.float32)).T)
    w3t_aug = np.zeros(((KT3 + 1) * P, C), dtype=np.float32)
    w3t_aug[:H3] = inputs["W3"].astype(np.float32).T
    w3t_aug[H3] = inputs["b3"].astype(np.float32)
    w3t_aug = bf(w3t_aug)

    def pack(v, mt):
        return np.ascontiguousarray(
            np.asarray(v, dtype=np.float32).reshape(mt, P).T
        )

    shared = {
        "w0t": w0t,
        "w1t": w1t,
        "w2t": w2t,
        "w3t": w3t_aug,
        "g0p": pack(inputs["g0"], MT0),
        "b0p": pack(inputs["beta0"], MT0),
        "g1p": pack(inputs["g1"], MT1),
        "b1p": pack(inputs["beta1"], MT1),
        "g2p": pack(inputs["g2"], MT2),
        "b2p": pack(inputs["beta2"], MT2),
    }
    in_maps = []
    for i in range(n_cores):
        xs = x[i * b_shard : (i + 1) * b_shard]  # [b_shard, D_IN]
        m = dict(shared)
        m["xT"] = bf(xs.T)  # [D_IN, b_shard]
        in_maps.append(m)
    return in_maps


_CACHE = {}


def _get_compiled(b_shard: int, n_cores: int):
    key = (b_shard, n_cores)
    if key not in _CACHE:
        _CACHE[key] = build(b_shard, n_cores)
    return _CACHE[key]


def kernel(**inputs) -> np.ndarray:
    b_shard = B_TOTAL // N_CORES
    nc = _get_compiled(b_shard, N_CORES)
    in_maps = prep_inputs(inputs, b_shard, N_CORES)
    res = run_bass_kernel_spmd(nc, in_maps, core_ids=list(range(N_CORES)))
    out = np.concatenate([r["out"] for r in res.results], axis=0)
    return out.astype(np.float32)


if __name__ == "__main__":
    data = np.load("/tmp/ref_data.npz")
    inputs = {k: data[k] for k in data.files if k != "expected"}
    expected = data["expected"]
    actual = kernel(**inputs)
    err = np.abs(actual - expected)
    print("max abs err:", err.max())
    print("absmax-rel:", err.max() / np.abs(expected).max())


# revision 13
# speedup vs baseline: 1.2101x; 1.2101x over previous
"""BinaryMLP (dense_mlp) Trainium2 kernel — 8-core data-parallel sync-BN.

Strategy:
  - Shard batch (4096) across 8 NeuronCores (512 rows each); replicate weights.
  - Activations live in SBUF transposed: [features -> partitions, batch -> free].
    BatchNorm batch stats are then free-axis reductions (VectorE / ACT accum).
  - Matmuls: lhsT = W.T tile (stationary), rhs = xT tile (moving), bf16 in,
    fp32 PSUM accumulation. Weight sign() / transposes / casts done host-side.
  - Sync-BN: per-feature sum / sumsq reduced locally, then one 32KB fp32
    AllReduce per BN layer across the 8 cores.
  - Final Linear flips layout back to [batch -> partitions, classes -> free] by
    using the h3 activation tiles as the stationary operand; log_softmax is a
    free-axis max/exp-accum/ln chain; b3 is folded in via an extra
    ones-row x b3-row contraction tile.
"""

import os
import sys

for _p in ("/opt/trn_rl_repo",):
    if _p not in sys.path and os.path.isdir(_p):
        sys.path.insert(0, _p)

import numpy as np
import ml_dtypes

import concourse.bass as bass
import concourse.mybir as mybir
import concourse.tile as tile
from concourse import bacc
from concourse.bass_utils import run_bass_kernel_spmd

AF = mybir.ActivationFunctionType
ALU = mybir.AluOpType
F32 = mybir.dt.float32
BF16 = mybir.dt.bfloat16
AX = mybir.AxisListType

NP_BF16 = ml_dtypes.bfloat16

P = 128
N_CORES = 8
B_TOTAL = 4096
D_IN = 4096
H1, H2, H3 = 4096, 4096, 2048
C = 1000
BN_EPS = 1e-5

KT0, MT0 = D_IN // P, H1 // P  # 32, 32
KT1, MT1 = H1 // P, H2 // P  # 32, 32
KT2, MT2 = H2 // P, H3 // P  # 32, 16
KT3 = H3 // P  # 16 (+1 aug tile for the bias)
MG = 4  # out-feature tiles per PSUM group (4 banks; 2 groups in flight)
KPAIR = 2  # k-tiles per weight-slab DMA


def build(b_shard: int, n_cores: int):
    """Build + compile the SPMD program for a per-core batch shard of b_shard."""
    assert b_shard % P == 0
    nb = b_shard // P  # batch tiles for the final layer
    n_batch_global = b_shard * n_cores
    rg = [list(range(n_cores))]

    nc = bacc.Bacc(
        "TRN2", target_bir_lowering=False, debug=False, num_devices=n_cores
    )

    xT = nc.dram_tensor("xT", [D_IN, b_shard], BF16, kind="ExternalInput").ap()
    w0t = nc.dram_tensor("w0t", [D_IN, H1], BF16, kind="ExternalInput").ap()
    w1t = nc.dram_tensor("w1t", [H1, H2], BF16, kind="ExternalInput").ap()
    w2t = nc.dram_tensor("w2t", [H2, H3], BF16, kind="ExternalInput").ap()
    # W3.T augmented with a b3 row (row H3) + zero padding to a full k-tile.
    w3t = nc.dram_tensor("w3t", [(KT3 + 1) * P, C], BF16, kind="ExternalInput").ap()
    g0p = nc.dram_tensor("g0p", [P, MT0], F32, kind="ExternalInput").ap()
    b0p = nc.dram_tensor("b0p", [P, MT0], F32, kind="ExternalInput").ap()
    g1p = nc.dram_tensor("g1p", [P, MT1], F32, kind="ExternalInput").ap()
    b1p = nc.dram_tensor("b1p", [P, MT1], F32, kind="ExternalInput").ap()
    g2p = nc.dram_tensor("g2p", [P, MT2], F32, kind="ExternalInput").ap()
    b2p = nc.dram_tensor("b2p", [P, MT2], F32, kind="ExternalInput").ap()
    out = nc.dram_tensor("out", [b_shard, C], F32, kind="ExternalOutput").ap()

    with tile.TileContext(nc) as tc:
        with (
            tc.tile_pool(name="big", bufs=1) as big,
            tc.tile_pool(name="wpool", bufs=6) as wpool,
            tc.tile_pool(name="psum", bufs=8, space="PSUM") as psum,
            tc.tile_pool(name="scratch", bufs=4) as scratch,
            tc.tile_pool(name="bn", bufs=8) as bnp,
            tc.tile_pool(name="small", bufs=24) as small,
            tc.tile_pool(name="dram", bufs=1, space="DRAM") as dram,
        ):
            # ---- persistent activation buffers -------------------------------
            xT_sb = big.tile([P, KT0, b_shard], BF16, name="xT_sb", tag="xT_sb")
            h1_sb = big.tile([P, MT0, b_shard], BF16, name="h1_sb", tag="h1_sb")
            h2_sb = big.tile([P, MT1, b_shard], BF16, name="h2_sb", tag="h2_sb")
            h3_sb = big.tile([P, MT2, b_shard], BF16, name="h3_sb", tag="h3_sb")
            ones_t = big.tile([P, b_shard], BF16, name="ones_t", tag="ones_t")

            nc.gpsimd.memset(ones_t[:], 0.0)
            nc.gpsimd.memset(ones_t[:1, :], 1.0)
            # chunked input load on the gpsimd (SWDGE) queue so it doesn't
            # queue behind weight-slab DMAs on sync; first matmuls start
            # after chunk 0 lands.
            xT_r = xT.rearrange("(ko p) b -> p ko b", p=P)
            XCH = KT0 // 4
            for c in range(4):
                nc.gpsimd.dma_start(
                    xT_sb[:, c * XCH : (c + 1) * XCH, :],
                    xT_r[:, c * XCH : (c + 1) * XCH, :],
                )

            # BN gamma/beta (host packed to [P, MT])
            gb = {}
            for nm, ap_, mt in (
                ("g0", g0p, MT0),
                ("b0", b0p, MT0),
                ("g1", g1p, MT1),
                ("b1", b1p, MT1),
                ("g2", g2p, MT2),
                ("b2", b2p, MT2),
            ):
                t = big.tile([P, mt], F32, name=f"{nm}_sb", tag=f"{nm}_sb")
                nc.gpsimd.dma_start(t[:], ap_)
                gb[nm] = t

            def mlp_layer(lidx, in_sb, kt, mt, w_dram, g_sb, b_sb, out_sb):
                """out_sb <- relu(bn(in_sb.T @ W.T)), transposed layout.

                Pipelined sync-BN: per-feature stats are all-reduced in two
                halves; AR-a overlaps the second half's matmuls and AR-b
                overlaps the NEXT layer's matmuls (whose k-loops are split so
                chunk-a input tiles are consumed first).
                """
                ngroups = mt // MG
                half_g = ngroups // 2
                half_m = mt // 2
                kh = kt // 2
                inv_n = 1.0 / float(n_batch_global)

                stats = [
                    big.tile(
                        [P, 2 * half_m], F32, name=f"stats{lidx}_{c}",
                        tag=f"stats{lidx}_{c}",
                    )
                    for c in range(2)
                ]
                gstats = [
                    big.tile(
                        [P, 2 * half_m], F32, name=f"gstats{lidx}_{c}",
                        tag=f"gstats{lidx}_{c}",
                    )
                    for c in range(2)
                ]
                arin = [
                    dram.tile(
                        [P, 2 * half_m], F32, name=f"arin{lidx}_{c}",
                        tag=f"arin{lidx}_{c}",
                    )
                    for c in range(2)
                ]
                arout = [
                    dram.tile(
                        [P, 2 * half_m], F32, name=f"arout{lidx}_{c}",
                        tag=f"arout{lidx}_{c}",
                    )
                    for c in range(2)
                ]
                s_sb = [
                    big.tile([P, half_m], F32, name=f"s{lidx}_{c}", tag=f"s{lidx}_{c}")
                    for c in range(2)
                ]
                t_sb = [
                    big.tile([P, half_m], F32, name=f"t{lidx}_{c}", tag=f"t{lidx}_{c}")
                    for c in range(2)
                ]

                ps_tiles = {}

                def emit_mms(g, k_lo, k_hi):
                    if g not in ps_tiles:
                        ps_tiles[g] = [
                            psum.tile(
                                [P, b_shard], F32, name=f"ps{lidx}_{g}_{j}", tag="ps"
                            )
                            for j in range(MG)
                        ]
                    ps = ps_tiles[g]
                    for kp in range(k_lo, k_hi, KPAIR):
                        slab = wpool.tile(
                            [P, KPAIR, MG * P], BF16,
                            name=f"w{lidx}_{g}_{kp}", tag="wslab",
                        )
                        nc.sync.dma_start(
                            slab[:],
                            w_dram[
                                kp * P : (kp + KPAIR) * P,
                                g * MG * P : (g + 1) * MG * P,
                            ].rearrange("(kk p) c -> p kk c", p=P),
                        )
                        for kk in range(KPAIR):
                            k = kp + kk
                            for j in range(MG):
                                nc.tensor.matmul(
                                    ps[j][:],
                                    slab[:, kk, j * P : (j + 1) * P],
                                    in_sb[:, k, :],
                                    start=(k == 0),
                                    stop=(k == kt - 1),
                                )

                def emit_stats(g):
                    c = g // half_g
                    st = stats[c]
                    for j in range(MG):
                        m = g * MG + j
                        ml = m - c * half_m
                        nc.scalar.activation(out_sb[:, m, :], ps_tiles[g][j][:], AF.Copy)
                        nc.vector.tensor_reduce(
                            st[:, ml : ml + 1], ps_tiles[g][j][:], axis=AX.X, op=ALU.add
                        )
                        sq = scratch.tile(
                            [P, b_shard], F32, name=f"sq{lidx}_{m}", tag="sq"
                        )
                        # HW: only one PSUM read per DVE inst, so square on ACT
                        nc.scalar.activation(
                            sq[:],
                            ps_tiles[g][j][:],
                            AF.Square,
                            accum_out=st[:, half_m + ml : half_m + ml + 1],
                        )

                def emit_ar(c):
                    nc.gpsimd.dma_start(arin[c][:], stats[c][:])
                    nc.gpsimd.collective_compute(
                        "AllReduce",
                        ALU.add,
                        replica_groups=rg,
                        ins=[arin[c].opt()],
                        outs=[arout[c].opt()],
                    )
                    nc.gpsimd.dma_start(gstats[c][:], arout[c][:])

                def emit_apply(c):
                    # s = g * rsqrt(var+eps); t = beta - mean*s, then in-place
                    # relu(h*s + t) for this chunk's feature tiles.
                    gs = gstats[c]
                    mean = bnp.tile([P, half_m], F32, name=f"mean{lidx}_{c}", tag="bn")
                    ex2 = bnp.tile([P, half_m], F32, name=f"ex2{lidx}_{c}", tag="bn")
                    m2 = bnp.tile([P, half_m], F32, name=f"m2{lidx}_{c}", tag="bn")
                    var = bnp.tile([P, half_m], F32, name=f"var{lidx}_{c}", tag="bn")
                    inv = bnp.tile([P, half_m], F32, name=f"inv{lidx}_{c}", tag="bn")
                    rstd = bnp.tile([P, half_m], F32, name=f"rstd{lidx}_{c}", tag="bn")
                    tmp = bnp.tile([P, half_m], F32, name=f"tmp{lidx}_{c}", tag="bn")
                    nc.scalar.activation(mean[:], gs[:, :half_m], AF.Copy, scale=inv_n)
                    nc.scalar.activation(ex2[:], gs[:, half_m:], AF.Copy, scale=inv_n)
                    nc.vector.tensor_mul(m2[:], mean[:], mean[:])
                    nc.vector.tensor_sub(var[:], ex2[:], m2[:])
                    nc.vector.tensor_scalar_add(var[:], var[:], BN_EPS)
                    nc.vector.reciprocal(inv[:], var[:])
                    nc.scalar.activation(rstd[:], inv[:], AF.Sqrt)
                    nc.vector.tensor_mul(
                        s_sb[c][:], rstd[:], g_sb[:, c * half_m : (c + 1) * half_m]
                    )
                    nc.vector.tensor_mul(tmp[:], mean[:], s_sb[c][:])
                    nc.vector.tensor_sub(
                        t_sb[c][:], b_sb[:, c * half_m : (c + 1) * half_m], tmp[:]
                    )
                    for ml in range(half_m):
                        m = c * half_m + ml
                        nc.scalar.activation(
                            out_sb[:, m, :],
                            out_sb[:, m, :],
                            AF.Relu,
                            bias=t_sb[c][:, ml : ml + 1],
                            scale=s_sb[c][:, ml : ml + 1],
                        )

                # groups 0,1: k-loop split so the first half only needs the
                # previous layer's chunk-a (covers that layer's AR-b latency)
                emit_mms(0, 0, kh)
                emit_mms(1, 0, kh)
                emit_mms(0, kh, kt)
                emit_stats(0)
                emit_mms(1, kh, kt)
                emit_stats(1)
                for g in range(2, half_g):
                    emit_mms(g, 0, kt)
                    emit_stats(g)
                emit_ar(0)  # chunk-a stats AR overlaps chunk-b matmuls
                apply_a_at = half_g + 1 if half_g > 2 else half_g
                for g in range(half_g, ngroups):
                    emit_mms(g, 0, kt)
                    emit_stats(g)
                    if g == apply_a_at:
                        emit_apply(0)
                emit_ar(1)
                emit_apply(1)

            mlp_layer(0, xT_sb, KT0, MT0, w0t, gb["g0"], gb["b0"], h1_sb)
            mlp_layer(1, h1_sb, KT1, MT1, w1t, gb["g1"], gb["b1"], h2_sb)

            # preload ALL final-layer weight slabs now — the DMAs run during
            # layer 2's compute and layer 3 then never waits on weight loads
            # gpsimd (SWDGE) queue: runs during layer 2 without delaying the
            # sync-queue weight-slab stream
            w3_sb = big.tile([P, KT3 + 1, C], BF16, name="w3_sb", tag="w3_sb")
            nc.gpsimd.dma_start(
                w3_sb[:], w3t.rearrange("(ko p) c -> p ko c", p=P)
            )

            mlp_layer(2, h2_sb, KT2, MT2, w2t, gb["g2"], gb["b2"], h3_sb)

            # ---- final Linear + log_softmax ---------------------------------
            # lhsT = h3 tile slice (stationary), rhs = preloaded W3.T slab
            # (moving). Output flips to [batch -> partitions, classes -> free].
            # k-loop split: first 8 k-tiles (layer 2's chunk-a) for every
            # batch tile first, covering layer 2's second stats-AR.
            half = (C + 1) // 2  # 500
            ka = KT3 // 2
            ps3 = [
                [
                    psum.tile([P, 512], F32, name=f"ps3_{b}_{h}", tag="ps")
                    for h in range(2)
                ]
                for b in range(nb)
            ]

            def l3_mms(b, k_lo, k_hi):
                for k in range(k_lo, k_hi):
                    lhsT = (
                        h3_sb[:, k, b * P : (b + 1) * P]
                        if k < KT3
                        else ones_t[:, b * P : (b + 1) * P]
                    )
                    for h in range(2):
                        nc.tensor.matmul(
                            ps3[b][h][:, : half],
                            lhsT,
                            w3_sb[:, k, h * half : (h + 1) * half],
                            start=(k == 0),
                            stop=(k == KT3),
                        )

            for b in range(nb):
                l3_mms(b, 0, ka)

            for b in range(nb):
                l3_mms(b, ka, KT3 + 1)
                p0 = ps3[b][0][:, :half]
                p1 = ps3[b][1][:, :half]
                m0 = small.tile([P, 1], F32, name=f"m0_{b}", tag="sm")
                m1 = small.tile([P, 1], F32, name=f"m1_{b}", tag="sm")
                nmax = small.tile([P, 1], F32, name=f"nmax_{b}", tag="sm")
                s0 = small.tile([P, 1], F32, name=f"s0_{b}", tag="sm")
                s1 = small.tile([P, 1], F32, name=f"s1_{b}", tag="sm")
                ssum = small.tile([P, 1], F32, name=f"ssum_{b}", tag="sm")
                lse = small.tile([P, 1], F32, name=f"lse_{b}", tag="sm")
                shift = small.tile([P, 1], F32, name=f"shift_{b}", tag="sm")

                nc.vector.tensor_reduce(m0[:], p0, axis=AX.X, op=ALU.max)
                nc.vector.tensor_reduce(m1[:], p1, axis=AX.X, op=ALU.max)
                nc.vector.tensor_max(m0[:], m0[:], m1[:])
                nc.vector.tensor_scalar_mul(nmax[:], m0[:], -1.0)
                e0 = scratch.tile([P, 512], F32, name=f"e0_{b}", tag="sq")
                e1 = scratch.tile([P, 512], F32, name=f"e1_{b}", tag="sq")
                nc.scalar.activation(
                    e0[:, :half], p0, AF.Exp, bias=nmax[:], scale=1.0, accum_out=s0[:]
                )
                nc.scalar.activation(
                    e1[:, :half], p1, AF.Exp, bias=nmax[:], scale=1.0, accum_out=s1[:]
                )
                nc.vector.tensor_add(ssum[:], s0[:], s1[:])
                nc.scalar.activation(lse[:], ssum[:], AF.Ln)
                nc.vector.tensor_sub(shift[:], nmax[:], lse[:])
                o0 = scratch.tile([P, 512], F32, name=f"o0_{b}", tag="sq")
                o1 = scratch.tile([P, 512], F32, name=f"o1_{b}", tag="sq")
                nc.scalar.activation(
                    o0[:, :half], p0, AF.Identity, bias=shift[:], scale=1.0
                )
                nc.scalar.activation(
                    o1[:, :half], p1, AF.Identity, bias=shift[:], scale=1.0
                )
                nc.sync.dma_start(out[b * P : (b + 1) * P, :half], o0[:, :half])
                nc.sync.dma_start(out[b * P : (b + 1) * P, half:C], o1[:, :half])

    nc.compile()
    return nc


def prep_inputs(inputs, b_shard: int, n_cores: int):
    """Host-side prep: shard x, transpose/cast weights, pack BN params."""
    x = np.ascontiguousarray(inputs["x"], dtype=np.float32)

    def bf(a):
        return np.ascontiguousarray(a).astype(NP_BF16)

    def sign_f32(w):
        return np.where(w >= 0, np.float32(1.0), np.float32(-1.0))

    w0t = bf(inputs["W0"].astype(np.float32).T)
    w1t = bf(sign_f32(np.asarray(inputs["Wb1"], dtype=np.float32)).T)
    w2t = bf(sign_f32(np.asarray(inputs["Wb2"], dtype=np.float32)).T)
    w3t_aug = np.zeros(((KT3 + 1) * P, C), dtype=np.float32)
    w3t_aug[:H3] = inputs["W3"].astype(np.float32).T
    w3t_aug[H3] = inputs["b3"].astype(np.float32)
    w3t_aug = bf(w3t_aug)

    def pack(v, mt):
        return np.ascontiguousarray(
            np.asarray(v, dtype=np.float32).reshape(mt, P).T
        )

    shared = {
        "w0t": w0t,
        "w1t": w1t,
        "w2t": w2t,
        "w3t": w3t_aug,
        "g0p": pack(inputs["g0"], MT0),
        "b0p": pack(inputs["beta0"], MT0),
        "g1p": pack(inputs["g1"], MT1),
        "b1p": pack(inputs["beta1"], MT1),
        "g2p": pack(inputs["g2"], MT2),
        "b2p": pack(inputs["beta2"], MT2),
    }
    in_maps = []
    for i in range(n_cores):
        xs = x[i * b_shard : (i + 1) * b_shard]  # [b_shard, D_IN]
        m = dict(shared)
        m["xT"] = bf(xs.T)  # [D_IN, b_shard]
        in_maps.append(m)
    return in_maps


_CACHE = {}


def _get_compiled(b_shard: int, n_cores: int):
    key = (b_shard, n_cores)
    if key not in _CACHE:
        _CACHE[key] = build(b_shard, n_cores)
    return _CACHE[key]


def kernel(**inputs) -> np.ndarray:
    b_shard = B_TOTAL // N_CORES
    nc = _get_compiled(b_shard, N_CORES)
    in_maps = prep_inputs(inputs, b_shard, N_CORES)
    res = run_bass_kernel_spmd(nc, in_maps, core_ids=list(range(N_CORES)))
    out = np.concatenate([r["out"] for r in res.results], axis=0)
    return out.astype(np.float32)


if __name__ == "__main__":
    data = np.load("/tmp/ref_data.npz")
    inputs = {k: data[k] for k in data.files if k != "expected"}
    expected = data["expected"]
    actual = kernel(**inputs)
    err = np.abs(actual - expected)
    print("max abs err:", err.max())
    print("absmax-rel:", err.max() / np.abs(expected).max())
